# revision 2
# baseline (speedup 1.0000x reference)
"""GPTNeoX attention block on 8 Trainium2 NeuronCores — v2.

Sharding: tensor-parallel over heads (2 heads/core) for QKV + attention,
then an AllToAll (bf16 payload) converts the head-sharded attention output
into a token-sharded layout so the dense output projection is fully local.

v2 over baseline: all-bf16 dataflow (same PE rate, half the DMA/collective
bytes, 2-4x DVE), causal diagonal trimming, per-batch dense phase so only
the last collective is exposed, wd preloaded to SBUF, software-pipelined
emission so PE doesn't stall on ACT/DVE chains.
"""
import sys

sys.path.insert(0, "/opt/trn_rl_repo")

import numpy as np
import ml_dtypes

import concourse.bass as bass
import concourse.mybir as mybir
import concourse.tile as tile
from concourse import bacc, bass_utils

F32 = mybir.dt.float32
BF16 = mybir.dt.bfloat16
Act = mybir.ActivationFunctionType
Alu = mybir.AluOpType

B = 2
S = 2048
H = 2048
NH = 16
HD = 128
RD = 32
ROPE_BASE = 10000.0
N_CORES = 8
HPC = NH // N_CORES          # heads per core = 2
SCALE = 1.0 / float(np.sqrt(HD))
NCH = S // 512               # 4 q-chunks of 512 per batch
NKT = S // 128               # 16 k-tiles of 128 per batch
TBLK = S // N_CORES          # 256 tokens per (batch, dest-core) block
DEPTH = 3                    # sc->av software pipeline depth
XT_BUFS = 14
EXP_BUFS = 4

_NC = {}


def _build(repeat=1):
    nc = bacc.Bacc("TRN2", target_bir_lowering=False, debug=False,
                   num_devices=N_CORES)

    xT = nc.dram_tensor("xT", [B, H, S], BF16, kind="ExternalInput")
    wqT = nc.dram_tensor("wqT", [H, HPC * HD], BF16, kind="ExternalInput")
    wkT = nc.dram_tensor("wkT", [H, HPC * HD], BF16, kind="ExternalInput")
    wvT = nc.dram_tensor("wvT", [H, HPC * HD], BF16, kind="ExternalInput")
    wdT = nc.dram_tensor("wdT", [H, H], BF16, kind="ExternalInput")
    bq_col = nc.dram_tensor("bq_col", [128, HPC], F32, kind="ExternalInput")
    bk_col = nc.dram_tensor("bk_col", [128, HPC], F32, kind="ExternalInput")
    bv_row2 = nc.dram_tensor("bv_row2", [1, 512], BF16, kind="ExternalInput")
    bd_row = nc.dram_tensor("bd_row", [1, H], BF16, kind="ExternalInput")
    cosT = nc.dram_tensor("cosT", [B, RD, S], BF16, kind="ExternalInput")
    sinT = nc.dram_tensor("sinT", [B, RD, S], BF16, kind="ExternalInput")
    kbias = nc.dram_tensor("kbias", [128, B * NKT], F32, kind="ExternalInput")
    rT = nc.dram_tensor("rT", [RD, RD], BF16, kind="ExternalInput")
    out = nc.dram_tensor("out", [B * TBLK, H], F32, kind="ExternalOutput")

    with tile.TileContext(nc) as tc:
        with tc.tile_pool(name="const", bufs=1) as cp, \
             tc.tile_pool(name="store", bufs=1) as st, \
             tc.tile_pool(name="work", bufs=2) as wk, \
             tc.tile_pool(name="ps", bufs=8, space="PSUM") as ps, \
             tc.tile_pool(name="dram", bufs=1, space="DRAM") as dram:

            # ---- constants (split loads so the first matmuls start fast) --
            wq_sb = cp.tile([128, NKT, HPC * HD], BF16, name="wq_sb")
            wk_sb = cp.tile([128, NKT, HPC * HD], BF16, name="wk_sb")
            wv_sb = cp.tile([128, NKT, HPC * HD], BF16, name="wv_sb")
            def _w_part(part):
                for w_sb, w_dram in ((wq_sb, wqT), (wk_sb, wkT), (wv_sb, wvT)):
                    src = w_dram.ap().rearrange("(n p) f -> p n f", p=128)
                    nc.sync.dma_start(w_sb[:, 4 * part:4 * (part + 1), :],
                                      src[:, 4 * part:4 * (part + 1), :])
            # parts are emitted interleaved with the first chunk's xt loads
            # (see emit_qkv) so the first matmul starts ~2us in
            weight_parts = [lambda p=p: _w_part(p) for p in range(4)]

            # small consts go on the Pool (SWDGE) DMA queue so they don't delay the
            # first x tiles on the SP queue
            cos_sb = cp.tile([RD, B, S], BF16, name="cos_sb")
            sin_sb = cp.tile([RD, B, S], BF16, name="sin_sb")
            nc.gpsimd.dma_start(cos_sb[:], cosT.ap().transpose([1, 0, 2]))
            nc.gpsimd.dma_start(sin_sb[:], sinT.ap().transpose([1, 0, 2]))

            kb_sb = cp.tile([128, B * NKT], F32, name="kb_sb")
            nc.gpsimd.dma_start(kb_sb[:], kbias.ap())
            bqc_sb = cp.tile([128, HPC], F32, name="bqc_sb")
            nc.gpsimd.dma_start(bqc_sb[:], bq_col.ap())
            bkc_sb = cp.tile([128, HPC], F32, name="bkc_sb")
            nc.gpsimd.dma_start(bkc_sb[:], bk_col.ap())
            bvr2_sb = cp.tile([1, 512], BF16, name="bvr2_sb")
            nc.gpsimd.dma_start(bvr2_sb[:], bv_row2.ap())
            bdr_sb = cp.tile([1, H], BF16, name="bdr_sb")
            nc.gpsimd.dma_start(bdr_sb[:], bd_row.ap())
            rT_sb = cp.tile([RD, RD], BF16, name="rT_sb")
            nc.gpsimd.dma_start(rT_sb[:], rT.ap())

            ones_f = wk.tile([128, 128], F32, tag="onesf", name="ones_f")
            nc.vector.memset(ones_f[:], 1.0)
            ones_col = cp.tile([128, 1], BF16, name="ones_col")
            nc.vector.tensor_copy(ones_col[:], ones_f[:, 0:1])
            ones_row = cp.tile([1, 128], BF16, name="ones_row")
            nc.vector.tensor_copy(ones_row[:], ones_f[0:1, :])

            # single [128,128] causal band mask: keep when p <= f
            mask_f = wk.tile([128, 128], F32, tag="onesf", name="mask_f")
            nc.gpsimd.affine_select(
                out=mask_f[:], in_=ones_f[:],
                compare_op=mybir.AluOpType.is_ge, fill=0.0,
                base=0, pattern=[[1, 128]], channel_multiplier=-1)
            mask128 = cp.tile([128, 128], BF16, name="mask128")
            nc.vector.tensor_copy(mask128[:], mask_f[:])

            # dense weights, preloaded to SBUF (loads issued later)
            wd_sb = cp.tile([128, NH, H], BF16, name="wd_sb")
            wd_src = wdT.ap().rearrange("(n p) f -> p n f", p=128)

            for _rep in range(repeat):
                pending = []

                def flush(n=10 ** 9):
                    for _ in range(min(n, len(pending))):
                        pending.pop(0)()

                a2a_outs = {}
                qs = {}
                ks = {}
                vs = {}

                # ================= QKV projection for batch b ===============
                def emit_qkv(b, rep=_rep):
                    sfx = f"{b}_{rep}"
                    qs[b] = [st.tile([128, S], BF16, tag=f"qs{f}_{b}",
                                     name=f"qs{f}_b{sfx}")
                             for f in range(HPC)]
                    ks[b] = [st.tile([128, S], BF16, tag=f"ks{f}_{b}",
                                     name=f"ks{f}_b{sfx}")
                             for f in range(HPC)]
                    vs[b] = st.tile([128, NKT * HPC * HD], BF16,
                                    tag=f"vs{b}", name=f"vs_b{sfx}")
                    for ch in range(NCH):
                        c0 = 512 * ch
                        q_ps = [ps.tile([128, 512], F32, tag="ps",
                                        name=f"qps{sfx}_{ch}_{f}")
                                for f in range(HPC)]
                        k_ps = [ps.tile([128, 512], F32, tag="ps",
                                        name=f"kps{sfx}_{ch}_{f}")
                                for f in range(HPC)]
                        v_ps = [ps.tile([128, 512], F32, tag="ps",
                                        name=f"vps{sfx}_{ch}_{u}")
                                for u in range(2)]
                        for ht in range(NKT):
                            if b == 0 and ch == 0 and ht % 4 == 0 \
                                    and weight_parts:
                                weight_parts.pop(0)()
                            xt = wk.tile([128, 512], BF16, tag="xt",
                                         name=f"xt{sfx}_{ch}_{ht}", bufs=XT_BUFS)
                            nc.sync.dma_start(
                                xt[:], xT.ap()[b, 128 * ht:128 * (ht + 1),
                                               c0:c0 + 512])
                            strt = (ht == 0)
                            stop = (ht == NKT - 1)
                            for f in range(HPC):
                                nc.tensor.matmul(
                                    q_ps[f][:],
                                    wq_sb[:, ht, 128 * f:128 * (f + 1)],
                                    xt[:], start=strt, stop=stop)
                                nc.tensor.matmul(
                                    k_ps[f][:],
                                    wk_sb[:, ht, 128 * f:128 * (f + 1)],
                                    xt[:], start=strt, stop=stop)
                            for t in range(4):
                                # start only on the bank's FIRST matmul: a
                                # second start would clear the whole bank's
                                # has_written bits, wiping the other half's
                                # ht=0 contribution
                                nc.tensor.matmul(
                                    v_ps[t // 2][:, 256 * (t % 2):
                                                 256 * (t % 2) + 256],
                                    xt[:, 128 * t:128 * (t + 1)],
                                    wv_sb[:, ht, :],
                                    start=(strt and t % 2 == 0), stop=False)
                            if ht in (6, 12):
                                flush(2)
                        # v bias matmuls (PE) first so v_ps stops early
                        for u in range(2):
                            nc.tensor.matmul(v_ps[u][:], ones_row[:],
                                             bvr2_sb[:], start=False,
                                             stop=True)
                        # q/k eviction with bias: q on ACT, k on DVE so the
                        # chunk-boundary eviction latency halves; emitted
                        # before the v evictions since next chunk's first
                        # matmuls recycle the q/k banks soonest
                        for tgt, src_ps, bias, nm in (
                                (qs[b], q_ps, bqc_sb, "q"),
                                (ks[b], k_ps, bkc_sb, "k")):
                            for f in range(HPC):
                                if nm == "q":
                                    nc.scalar.activation(
                                        tgt[f][:, c0:c0 + 512], src_ps[f][:],
                                        Act.Identity, bias=bias[:, f:f + 1],
                                        scale=1.0)
                                else:
                                    nc.vector.tensor_scalar_add(
                                        tgt[f][:, c0:c0 + 512], src_ps[f][:],
                                        bias[:, f:f + 1])

                                def rope(tgt=tgt, f=f, c0=c0, b=b, nm=nm,
                                         sfx=sfx):
                                    rowq = tgt[f][0:RD, c0:c0 + 512]
                                    rot = ps.tile(
                                        [RD, 512], F32, tag="ps",
                                        name=f"rot{nm}{sfx}_{f}_{c0}")
                                    nc.tensor.matmul(rot[:], rT_sb[:], rowq,
                                                     start=True, stop=True)
                                    t1 = wk.tile(
                                        [RD, 512], BF16, tag="rt1",
                                        name=f"t1{nm}{sfx}_{f}_{c0}", bufs=3)
                                    nc.vector.tensor_mul(
                                        t1[:], rowq,
                                        cos_sb[:, b, c0:c0 + 512])
                                    t2 = wk.tile(
                                        [RD, 512], BF16, tag="rt2",
                                        name=f"t2{nm}{sfx}_{f}_{c0}", bufs=3)
                                    nc.vector.tensor_mul(
                                        t2[:], rot[:],
                                        sin_sb[:, b, c0:c0 + 512])
                                    nc.vector.tensor_add(rowq, t1[:], t2[:])
                                pending.append(rope)
                        # v eviction (DVE), after q/k evictions; DVE is
                        # light during QKV and this keeps ACT free for the
                        # next head's exps
                        for u in range(2):
                            kt = 4 * ch + 2 * u
                            nc.vector.tensor_copy(
                                vs[b][:, HPC * HD * kt:HPC * HD * (kt + 2)],
                                v_ps[u][:])

                # ================= attention for (b, hl) ====================
                def emit_head(b, hl, rep=_rep):
                    sfx = f"{b}_{hl}_{rep}"
                    q_st, k_st = qs[b][hl], ks[b][hl]
                    a2a_in = dram.tile([N_CORES, 128, TBLK], BF16,
                                       tag=f"a2a_in{b}{hl}",
                                       name=f"a2a_in{sfx}")
                    a2a_out = dram.tile([N_CORES, 128, TBLK], BF16,
                                        tag=f"a2a_out{b}{hl}",
                                        name=f"a2a_out{sfx}")
                    a2a_outs[(b, hl)] = a2a_out
                    steps = []   # sc->avden pipeline carried ACROSS j
                    for j in range(NCH):
                        nvalid = 4 * j + 4
                        attn_ps = ps.tile([128, 512], F32, tag="ps",
                                          name=f"aps{sfx}_{j}")
                        den_ps = ps.tile([1, 512], F32, tag="ps",
                                         name=f"dps{sfx}_{j}")
                        for i in range(nvalid):
                            r = i - 4 * j
                            o0 = 128 * r if r >= 0 else 0
                            sc = ps.tile([128, 512], F32, tag="ps",
                                         name=f"sc{sfx}_{j}_{i}")
                            nc.tensor.matmul(
                                sc[:, o0:512],
                                k_st[:, 128 * i:128 * (i + 1)],
                                q_st[:, 512 * j + o0:512 * (j + 1)],
                                start=True, stop=True)
                            ex = wk.tile([128, 512], BF16, tag="exp",
                                         name=f"ex{sfx}_{j}_{i}", bufs=EXP_BUFS)
                            nc.scalar.activation(
                                ex[:, o0:512], sc[:, o0:512], Act.Exp,
                                bias=kb_sb[:, NKT * b + i:NKT * b + i + 1],
                                scale=SCALE)
                            if r >= 0:
                                nc.vector.tensor_mul(
                                    ex[:, o0:o0 + 128], ex[:, o0:o0 + 128],
                                    mask128[:])

                            def avden(i=i, o0=o0, ex=ex, attn_ps=attn_ps,
                                      den_ps=den_ps, b=b, hl=hl,
                                      nvalid=nvalid):
                                nc.tensor.matmul(
                                    attn_ps[:, o0:512],
                                    vs[b][:, HPC * HD * i + HD * hl:
                                           HPC * HD * i + HD * (hl + 1)],
                                    ex[:, o0:512], start=(i == 0),
                                    stop=(i == nvalid - 1))
                                nc.tensor.matmul(
                                    den_ps[:, o0:512], ones_col[:],
                                    ex[:, o0:512], start=(i == 0),
                                    stop=(i == nvalid - 1))
                            steps.append(avden)
                            if len(steps) > DEPTH:
                                steps.pop(0)()
                            # norm(j-1) is only safe to flush once its stop
                            # avden has been emitted, i.e. at i >= DEPTH
                            if i == DEPTH or i == min(DEPTH + 2, nvalid - 1):
                                flush(1)

                        def norm(j=j, attn_ps=attn_ps, den_ps=den_ps,
                                 a2a_in=a2a_in, b=b, hl=hl, sfx=sfx):
                            dr = wk.tile([1, 512], F32, tag="dr",
                                         name=f"dr{sfx}_{j}", bufs=2)
                            nc.vector.reciprocal(dr[:], den_ps[:])
                            drr = wk.tile([1, 512], BF16, tag="drr",
                                          name=f"drr{sfx}_{j}", bufs=3)
                            nc.vector.tensor_copy(drr[:], dr[:])
                            # broadcast 1/den across partitions on the idle
                            # GPSIMD instead of a PE matmul
                            bcs = wk.tile([128, 512], BF16, tag="bcs",
                                          name=f"bcs{sfx}_{j}", bufs=3)
                            nc.gpsimd.partition_broadcast(bcs[:], drr[:])
                            at = wk.tile([128, 512], BF16, tag="at",
                                         name=f"at{sfx}_{j}", bufs=3)
                            nc.vector.scalar_tensor_tensor(
                                at[:], attn_ps[:], 1.0, bcs[:],
                                Alu.mult, Alu.mult)
                            for e in range(2):
                                nc.sync.dma_start(
                                    a2a_in[2 * j + e, :, :],
                                    at[:, TBLK * e:TBLK * (e + 1)])
                        pending.append(norm)
                    for s_fn in steps:
                        s_fn()
                    return a2a_in

                def head_phase(b, hl):
                    a2a_in = emit_head(b, hl)
                    flush()
                    nc.gpsimd.collective_compute(
                        "AllToAll", mybir.AluOpType.bypass,
                        replica_groups=[list(range(N_CORES))],
                        ins=[a2a_in.opt()],
                        outs=[a2a_outs[(b, hl)].opt()])

                # ---- phase emission order ---------------------------------
                emit_qkv(0)
                # wd preload on the Pool (SWDGE) DMA queue: runs during b0 attention
                # without delaying xt loads or a2a stores
                for g in range(NH):
                    nc.gpsimd.dma_start(wd_sb[:, g, :], wd_src[:, g, :])
                head_phase(0, 0)
                head_phase(0, 1)
                emit_qkv(1)
                head_phase(1, 0)
                head_phase(1, 1)
                flush()

                # ================= dense output projection =================
                # bank-outer / g-inner so each PSUM bank finishes early and
                # its bias+eviction pipelines under the next bank's matmuls
                for b in range(B):
                    ag_tiles = {}
                    for hl in range(HPC):
                        for r in range(N_CORES):
                            ag = wk.tile([128, TBLK], BF16, tag="ag",
                                         name=f"ag{b}_{hl}_{r}_{_rep}",
                                         bufs=16)
                            nc.sync.dma_start(
                                ag[:], a2a_outs[(b, hl)][r, :, :])
                            ag_tiles[(hl, r)] = ag
                    for oc in range(4):
                        for t in range(2):
                            opst = ps.tile([128, 512], F32, tag="ps",
                                           name=f"ops{b}_{oc}_{t}_{_rep}")
                            for hl in range(HPC):
                                for r in range(N_CORES):
                                    g = HPC * r + hl
                                    nc.tensor.matmul(
                                        opst[:],
                                        ag_tiles[(hl, r)][:,
                                                          128 * t:
                                                          128 * (t + 1)],
                                        wd_sb[:, g, 512 * oc:512 * (oc + 1)],
                                        start=(hl == 0 and r == 0),
                                        stop=False)
                            nc.tensor.matmul(
                                opst[:], ones_row[:],
                                bdr_sb[:, 512 * oc:512 * (oc + 1)],
                                start=False, stop=True)
                            os_t = wk.tile([128, 512], F32, tag="os",
                                           name=f"os{b}_{oc}_{t}_{_rep}",
                                           bufs=2)
                            nc.scalar.activation(os_t[:], opst[:], Act.Copy)
                            nc.sync.dma_start(
                                out.ap()[TBLK * b + 128 * t:
                                         TBLK * b + 128 * (t + 1),
                                         512 * oc:512 * (oc + 1)],
                                os_t[:])
    nc.compile()
    return nc


def _get_nc(repeat=1):
    if repeat not in _NC:
        _NC[repeat] = _build(repeat)
    return _NC[repeat]


def _host_prep(hidden_states, attention_mask, position_ids, W_qkv, b_qkv,
               W_dense, b_dense):
    bf16 = ml_dtypes.bfloat16
    x = np.asarray(hidden_states, dtype=np.float32)
    am = np.asarray(attention_mask, dtype=np.float32)
    pos = np.asarray(position_ids)
    W_qkv = np.asarray(W_qkv, dtype=np.float32)
    b_qkv = np.asarray(b_qkv, dtype=np.float32)
    W_dense = np.asarray(W_dense, dtype=np.float32)
    b_dense = np.asarray(b_dense, dtype=np.float32)

    xT = np.ascontiguousarray(np.transpose(x, (0, 2, 1))).astype(bf16)
    wdT = np.ascontiguousarray(W_dense.T).astype(bf16)
    bd_row = np.ascontiguousarray(b_dense[None, :]).astype(bf16)

    inv_freq = (1.0 / (ROPE_BASE ** (np.arange(0, RD, 2, dtype=np.float32)
                                     / RD))).astype(np.float32)
    cosT = np.empty((B, RD, S), np.float32)
    sinT = np.empty((B, RD, S), np.float32)
    for b in range(B):
        freqs = pos[b].astype(np.float32)[:, None] * inv_freq[None, :]
        emb = np.concatenate([freqs, freqs], axis=1)        # (S, RD)
        cosT[b] = np.cos(emb).T
        sinT[b] = np.sin(emb).T
    cosT = cosT.astype(bf16)
    sinT = sinT.astype(bf16)

    kbias = np.empty((128, B * NKT), np.float32)
    for b in range(B):
        kbias[:, NKT * b:NKT * (b + 1)] = am[b, 0, 0].reshape(NKT, 128).T

    rT = np.zeros((RD, RD), np.float32)
    half = RD // 2
    rT[np.arange(half), np.arange(half) + half] = 1.0
    rT[np.arange(half) + half, np.arange(half)] = -1.0
    rT = rT.astype(bf16)

    in_maps = []
    for c in range(N_CORES):
        heads = [HPC * c + hl for hl in range(HPC)]
        wq = np.concatenate([W_qkv[384 * g:384 * g + 128] for g in heads])
        wkk = np.concatenate([W_qkv[384 * g + 128:384 * g + 256]
                              for g in heads])
        wv = np.concatenate([W_qkv[384 * g + 256:384 * g + 384]
                             for g in heads])
        bq_col = np.stack([b_qkv[384 * g:384 * g + 128] for g in heads],
                          axis=1)
        bk_col = np.stack([b_qkv[384 * g + 128:384 * g + 256] for g in heads],
                          axis=1)
        bv = np.concatenate([b_qkv[384 * g + 256:384 * g + 384]
                             for g in heads])
        bv_row2 = np.concatenate([bv, bv])[None, :]   # duplicated for packed
        in_maps.append({
            "xT": xT,
            "wqT": np.ascontiguousarray(wq.T).astype(bf16),
            "wkT": np.ascontiguousarray(wkk.T).astype(bf16),
            "wvT": np.ascontiguousarray(wv.T).astype(bf16),
            "wdT": wdT,
            "bq_col": np.ascontiguousarray(bq_col),
            "bk_col": np.ascontiguousarray(bk_col),
            "bv_row2": np.ascontiguousarray(bv_row2).astype(bf16),
            "bd_row": bd_row,
            "cosT": cosT,
            "sinT": sinT,
            "kbias": kbias,
            "rT": rT,
        })
    return in_maps


def run_sharded(trace=False, **inputs):
    """Run the bass kernel; returns (full_output, BassKernelResults)."""
    import time as _time
    nc = _get_nc()
    in_maps = _host_prep(**inputs)
    last_err = None
    for attempt in range(4):
        try:
            res = bass_utils.run_bass_kernel_spmd(
                nc, in_maps, core_ids=list(range(N_CORES)), trace=trace)
            break
        except Exception as e:  # device occasionally wedged by a prior session
            msg = str(e)
            last_err = e
            if ("UNAVAILABLE" in msg or "UNRECOVERABLE" in msg
                    or "unrecoverable" in msg):
                _time.sleep(5.0 * (attempt + 1))
                continue
            raise
    else:
        raise last_err
    full = np.empty((B, S, H), np.float32)
    for c in range(N_CORES):
        shard = res.results[c]["out"]
        for b in range(B):
            full[b, TBLK * c:TBLK * (c + 1)] = shard[TBLK * b:TBLK * (b + 1)]
    return full, res


def kernel(**inputs):
    full, _ = run_sharded(trace=False, **inputs)
    return full


# revision 3
# speedup vs baseline: 2.1834x; 2.1834x over previous
"""GPTNeoX attention block on 8 Trainium2 NeuronCores — v2.

Sharding: tensor-parallel over heads (2 heads/core) for QKV + attention,
then an AllToAll (bf16 payload) converts the head-sharded attention output
into a token-sharded layout so the dense output projection is fully local.

v2 over baseline: all-bf16 dataflow (same PE rate, half the DMA/collective
bytes, 2-4x DVE), causal diagonal trimming, per-batch dense phase so only
the last collective is exposed, wd preloaded to SBUF, software-pipelined
emission so PE doesn't stall on ACT/DVE chains.
"""
import sys

sys.path.insert(0, "/opt/trn_rl_repo")

import numpy as np
import ml_dtypes

import concourse.bass as bass
import concourse.mybir as mybir
import concourse.tile as tile
from concourse import bacc, bass_utils

F32 = mybir.dt.float32
BF16 = mybir.dt.bfloat16
Act = mybir.ActivationFunctionType
Alu = mybir.AluOpType

B = 2
S = 2048
H = 2048
NH = 16
HD = 128
RD = 32
ROPE_BASE = 10000.0
N_CORES = 8
HPC = NH // N_CORES          # heads per core = 2
SCALE = 1.0 / float(np.sqrt(HD))
NCH = S // 512               # 4 q-chunks of 512 per batch
NKT = S // 128               # 16 k-tiles of 128 per batch
TBLK = S // N_CORES          # 256 tokens per (batch, dest-core) block
DEPTH = 4                    # sc->av software pipeline depth
XT_BUFS = 14
EXP_BUFS = 4

_NC = {}


def _build(repeat=1):
    nc = bacc.Bacc("TRN2", target_bir_lowering=False, debug=False,
                   num_devices=N_CORES)

    xT = nc.dram_tensor("xT", [B, H, S], BF16, kind="ExternalInput")
    wqT = nc.dram_tensor("wqT", [H, HPC * HD], BF16, kind="ExternalInput")
    wkT = nc.dram_tensor("wkT", [H, HPC * HD], BF16, kind="ExternalInput")
    wvT = nc.dram_tensor("wvT", [H, HPC * HD], BF16, kind="ExternalInput")
    wdT = nc.dram_tensor("wdT", [H, H], BF16, kind="ExternalInput")
    bq_col = nc.dram_tensor("bq_col", [128, HPC], F32, kind="ExternalInput")
    bk_col = nc.dram_tensor("bk_col", [128, HPC], F32, kind="ExternalInput")
    bv_row2 = nc.dram_tensor("bv_row2", [1, 512], BF16, kind="ExternalInput")
    bd_row = nc.dram_tensor("bd_row", [1, H], BF16, kind="ExternalInput")
    cosT = nc.dram_tensor("cosT", [B, RD, S], BF16, kind="ExternalInput")
    sinT = nc.dram_tensor("sinT", [B, RD, S], BF16, kind="ExternalInput")
    kbias = nc.dram_tensor("kbias", [128, B * NKT], F32, kind="ExternalInput")
    rT = nc.dram_tensor("rT", [RD, RD], BF16, kind="ExternalInput")
    out = nc.dram_tensor("out", [B * TBLK, H], F32, kind="ExternalOutput")

    with tile.TileContext(nc) as tc:
        with tc.tile_pool(name="const", bufs=1) as cp, \
             tc.tile_pool(name="store", bufs=1) as st, \
             tc.tile_pool(name="work", bufs=2) as wk, \
             tc.tile_pool(name="ps", bufs=8, space="PSUM") as ps, \
             tc.tile_pool(name="dram", bufs=1, space="DRAM") as dram:

            # ---- constants (split loads so the first matmuls start fast) --
            wq_sb = cp.tile([128, NKT, HPC * HD], BF16, name="wq_sb")
            wk_sb = cp.tile([128, NKT, HPC * HD], BF16, name="wk_sb")
            wv_sb = cp.tile([128, NKT, HPC * HD], BF16, name="wv_sb")
            def _w_part(part):
                for w_sb, w_dram in ((wq_sb, wqT), (wk_sb, wkT), (wv_sb, wvT)):
                    src = w_dram.ap().rearrange("(n p) f -> p n f", p=128)
                    nc.sync.dma_start(w_sb[:, 4 * part:4 * (part + 1), :],
                                      src[:, 4 * part:4 * (part + 1), :])
            # parts are emitted interleaved with the first chunk's xt loads
            # (see emit_qkv) so the first matmul starts ~2us in
            weight_parts = [lambda p=p: _w_part(p) for p in range(4)]

            # small consts go on the Pool (SWDGE) DMA queue so they don't delay the
            # first x tiles on the SP queue
            cos_sb = cp.tile([RD, B, S], BF16, name="cos_sb")
            sin_sb = cp.tile([RD, B, S], BF16, name="sin_sb")
            nc.gpsimd.dma_start(cos_sb[:], cosT.ap().transpose([1, 0, 2]))
            nc.gpsimd.dma_start(sin_sb[:], sinT.ap().transpose([1, 0, 2]))

            kb_sb = cp.tile([128, B * NKT], F32, name="kb_sb")
            nc.gpsimd.dma_start(kb_sb[:], kbias.ap())
            bqc_sb = cp.tile([128, HPC], F32, name="bqc_sb")
            nc.gpsimd.dma_start(bqc_sb[:], bq_col.ap())
            bkc_sb = cp.tile([128, HPC], F32, name="bkc_sb")
            nc.gpsimd.dma_start(bkc_sb[:], bk_col.ap())
            bvr2_sb = cp.tile([1, 512], BF16, name="bvr2_sb")
            nc.gpsimd.dma_start(bvr2_sb[:], bv_row2.ap())
            bdr_sb = cp.tile([1, H], BF16, name="bdr_sb")
            nc.gpsimd.dma_start(bdr_sb[:], bd_row.ap())
            rT_sb = cp.tile([RD, RD], BF16, name="rT_sb")
            nc.gpsimd.dma_start(rT_sb[:], rT.ap())

            ones_f = wk.tile([128, 128], F32, tag="onesf", name="ones_f")
            nc.vector.memset(ones_f[:], 1.0)
            ones_col = cp.tile([128, 1], BF16, name="ones_col")
            nc.vector.tensor_copy(ones_col[:], ones_f[:, 0:1])
            ones_row = cp.tile([1, 128], BF16, name="ones_row")
            nc.vector.tensor_copy(ones_row[:], ones_f[0:1, :])

            # single [128,128] causal band mask: keep when p <= f
            mask_f = wk.tile([128, 128], F32, tag="onesf", name="mask_f")
            nc.gpsimd.affine_select(
                out=mask_f[:], in_=ones_f[:],
                compare_op=mybir.AluOpType.is_ge, fill=0.0,
                base=0, pattern=[[1, 128]], channel_multiplier=-1)
            mask128 = cp.tile([128, 128], BF16, name="mask128")
            nc.vector.tensor_copy(mask128[:], mask_f[:])

            # dense weights, preloaded to SBUF (loads issued later)
            wd_sb = cp.tile([128, NH, H], BF16, name="wd_sb")
            wd_src = wdT.ap().rearrange("(n p) f -> p n f", p=128)

            for _rep in range(repeat):
                pending = []

                def flush(n=10 ** 9):
                    for _ in range(min(n, len(pending))):
                        pending.pop(0)()

                a2a_outs = {}
                qs = {}
                ks = {}
                vs = {}

                # ================= QKV projection for batch b ===============
                def emit_qkv(b, rep=_rep):
                    sfx = f"{b}_{rep}"
                    qs[b] = [st.tile([128, S], BF16, tag=f"qs{f}_{b}",
                                     name=f"qs{f}_b{sfx}")
                             for f in range(HPC)]
                    ks[b] = [st.tile([128, S], BF16, tag=f"ks{f}_{b}",
                                     name=f"ks{f}_b{sfx}")
                             for f in range(HPC)]
                    vs[b] = st.tile([128, NKT * HPC * HD], BF16,
                                    tag=f"vs{b}", name=f"vs_b{sfx}")
                    for ch in range(NCH):
                        c0 = 512 * ch
                        q_ps = [ps.tile([128, 512], F32, tag="ps",
                                        name=f"qps{sfx}_{ch}_{f}")
                                for f in range(HPC)]
                        k_ps = [ps.tile([128, 512], F32, tag="ps",
                                        name=f"kps{sfx}_{ch}_{f}")
                                for f in range(HPC)]
                        v_ps = [ps.tile([128, 512], F32, tag="ps",
                                        name=f"vps{sfx}_{ch}_{u}")
                                for u in range(2)]
                        for ht in range(NKT):
                            if b == 0 and ch == 0 and ht % 4 == 0 \
                                    and weight_parts:
                                weight_parts.pop(0)()
                            xt = wk.tile([128, 512], BF16, tag="xt",
                                         name=f"xt{sfx}_{ch}_{ht}", bufs=XT_BUFS)
                            nc.sync.dma_start(
                                xt[:], xT.ap()[b, 128 * ht:128 * (ht + 1),
                                               c0:c0 + 512])
                            strt = (ht == 0)
                            stop = (ht == NKT - 1)
                            for f in range(HPC):
                                nc.tensor.matmul(
                                    q_ps[f][:],
                                    wq_sb[:, ht, 128 * f:128 * (f + 1)],
                                    xt[:], start=strt, stop=stop)
                                nc.tensor.matmul(
                                    k_ps[f][:],
                                    wk_sb[:, ht, 128 * f:128 * (f + 1)],
                                    xt[:], start=strt, stop=stop)
                            for t in range(4):
                                # start only on the bank's FIRST matmul: a
                                # second start would clear the whole bank's
                                # has_written bits, wiping the other half's
                                # ht=0 contribution
                                nc.tensor.matmul(
                                    v_ps[t // 2][:, 256 * (t % 2):
                                                 256 * (t % 2) + 256],
                                    xt[:, 128 * t:128 * (t + 1)],
                                    wv_sb[:, ht, :],
                                    start=(strt and t % 2 == 0), stop=False)
                            if ht in (6, 12):
                                flush(2)
                        # v bias matmuls (PE) first so v_ps stops early
                        for u in range(2):
                            nc.tensor.matmul(v_ps[u][:], ones_row[:],
                                             bvr2_sb[:], start=False,
                                             stop=True)
                        # q/k eviction with bias: q on ACT, k on DVE so the
                        # chunk-boundary eviction latency halves; emitted
                        # before the v evictions since next chunk's first
                        # matmuls recycle the q/k banks soonest
                        for tgt, src_ps, bias, nm in (
                                (qs[b], q_ps, bqc_sb, "q"),
                                (ks[b], k_ps, bkc_sb, "k")):
                            for f in range(HPC):
                                if nm == "q":
                                    nc.scalar.activation(
                                        tgt[f][:, c0:c0 + 512], src_ps[f][:],
                                        Act.Identity, bias=bias[:, f:f + 1],
                                        scale=1.0)
                                else:
                                    nc.vector.tensor_scalar_add(
                                        tgt[f][:, c0:c0 + 512], src_ps[f][:],
                                        bias[:, f:f + 1])

                                def rope(tgt=tgt, f=f, c0=c0, b=b, nm=nm,
                                         sfx=sfx):
                                    rowq = tgt[f][0:RD, c0:c0 + 512]
                                    rot = ps.tile(
                                        [RD, 512], F32, tag="ps",
                                        name=f"rot{nm}{sfx}_{f}_{c0}")
                                    nc.tensor.matmul(rot[:], rT_sb[:], rowq,
                                                     start=True, stop=True)
                                    t1 = wk.tile(
                                        [RD, 512], BF16, tag="rt1",
                                        name=f"t1{nm}{sfx}_{f}_{c0}", bufs=3)
                                    nc.vector.tensor_mul(
                                        t1[:], rowq,
                                        cos_sb[:, b, c0:c0 + 512])
                                    t2 = wk.tile(
                                        [RD, 512], BF16, tag="rt2",
                                        name=f"t2{nm}{sfx}_{f}_{c0}", bufs=3)
                                    nc.vector.tensor_mul(
                                        t2[:], rot[:],
                                        sin_sb[:, b, c0:c0 + 512])
                                    nc.vector.tensor_add(rowq, t1[:], t2[:])
                                pending.append(rope)
                        # v eviction (DVE), after q/k evictions; DVE is
                        # light during QKV and this keeps ACT free for the
                        # next head's exps
                        for u in range(2):
                            kt = 4 * ch + 2 * u
                            nc.vector.tensor_copy(
                                vs[b][:, HPC * HD * kt:HPC * HD * (kt + 2)],
                                v_ps[u][:])

                # ================= attention for (b, hl) ====================
                def emit_head(b, hl, rep=_rep):
                    sfx = f"{b}_{hl}_{rep}"
                    q_st, k_st = qs[b][hl], ks[b][hl]
                    a2a_in = dram.tile([N_CORES, 128, TBLK], BF16,
                                       tag=f"a2a_in{b}{hl}",
                                       name=f"a2a_in{sfx}")
                    a2a_out = dram.tile([N_CORES, 128, TBLK], BF16,
                                        tag=f"a2a_out{b}{hl}",
                                        name=f"a2a_out{sfx}")
                    a2a_outs[(b, hl)] = a2a_out
                    steps = []   # sc->avden pipeline carried ACROSS j
                    for j in range(NCH):
                        nvalid = 4 * j + 4
                        attn_ps = ps.tile([128, 512], F32, tag="ps",
                                          name=f"aps{sfx}_{j}")
                        den_ps = ps.tile([1, 512], F32, tag="ps",
                                         name=f"dps{sfx}_{j}")
                        for i in range(nvalid):
                            r = i - 4 * j
                            o0 = 128 * r if r >= 0 else 0
                            sc = ps.tile([128, 512], F32, tag="ps",
                                         name=f"sc{sfx}_{j}_{i}")
                            nc.tensor.matmul(
                                sc[:, o0:512],
                                k_st[:, 128 * i:128 * (i + 1)],
                                q_st[:, 512 * j + o0:512 * (j + 1)],
                                start=True, stop=True)
                            ex = wk.tile([128, 512], BF16, tag="exp",
                                         name=f"ex{sfx}_{j}_{i}", bufs=EXP_BUFS)
                            nc.scalar.activation(
                                ex[:, o0:512], sc[:, o0:512], Act.Exp,
                                bias=kb_sb[:, NKT * b + i:NKT * b + i + 1],
                                scale=SCALE)
                            if r >= 0:
                                nc.vector.tensor_mul(
                                    ex[:, o0:o0 + 128], ex[:, o0:o0 + 128],
                                    mask128[:])

                            def avden(i=i, o0=o0, ex=ex, attn_ps=attn_ps,
                                      den_ps=den_ps, b=b, hl=hl,
                                      nvalid=nvalid):
                                nc.tensor.matmul(
                                    attn_ps[:, o0:512],
                                    vs[b][:, HPC * HD * i + HD * hl:
                                           HPC * HD * i + HD * (hl + 1)],
                                    ex[:, o0:512], start=(i == 0),
                                    stop=(i == nvalid - 1))
                                nc.tensor.matmul(
                                    den_ps[:, o0:512], ones_col[:],
                                    ex[:, o0:512], start=(i == 0),
                                    stop=(i == nvalid - 1))
                            steps.append(avden)
                            if len(steps) > DEPTH:
                                steps.pop(0)()
                            # norm(j-1) is only safe to flush once its stop
                            # avden has been emitted, i.e. at i >= DEPTH
                            if i == DEPTH or i == min(DEPTH + 2, nvalid - 1):
                                flush(1)

                        def norm(j=j, attn_ps=attn_ps, den_ps=den_ps,
                                 a2a_in=a2a_in, b=b, hl=hl, sfx=sfx):
                            dr = wk.tile([1, 512], F32, tag="dr",
                                         name=f"dr{sfx}_{j}", bufs=2)
                            nc.vector.reciprocal(dr[:], den_ps[:])
                            drr = wk.tile([1, 512], BF16, tag="drr",
                                          name=f"drr{sfx}_{j}", bufs=3)
                            nc.vector.tensor_copy(drr[:], dr[:])
                            # broadcast 1/den across partitions on the idle
                            # GPSIMD instead of a PE matmul
                            bcs = wk.tile([128, 512], BF16, tag="bcs",
                                          name=f"bcs{sfx}_{j}", bufs=3)
                            nc.gpsimd.partition_broadcast(bcs[:], drr[:])
                            at = wk.tile([128, 512], BF16, tag="at",
                                         name=f"at{sfx}_{j}", bufs=3)
                            nc.vector.scalar_tensor_tensor(
                                at[:], attn_ps[:], 1.0, bcs[:],
                                Alu.mult, Alu.mult)
                            for e in range(2):
                                nc.sync.dma_start(
                                    a2a_in[2 * j + e, :, :],
                                    at[:, TBLK * e:TBLK * (e + 1)])
                        pending.append(norm)
                    for s_fn in steps:
                        s_fn()
                    return a2a_in

                def head_phase(b, hl):
                    a2a_in = emit_head(b, hl)
                    flush()
                    nc.gpsimd.collective_compute(
                        "AllToAll", mybir.AluOpType.bypass,
                        replica_groups=[list(range(N_CORES))],
                        ins=[a2a_in.opt()],
                        outs=[a2a_outs[(b, hl)].opt()])

                # ---- phase emission order ---------------------------------
                emit_qkv(0)
                # wd preload on the Pool (SWDGE) DMA queue: runs during b0 attention
                # without delaying xt loads or a2a stores
                for g in range(NH):
                    nc.gpsimd.dma_start(wd_sb[:, g, :], wd_src[:, g, :])
                head_phase(0, 0)
                head_phase(0, 1)
                emit_qkv(1)
                head_phase(1, 0)
                head_phase(1, 1)
                flush()

                # ================= dense output projection =================
                # bank-outer / g-inner so each PSUM bank finishes early and
                # its bias+eviction pipelines under the next bank's matmuls
                for b in range(B):
                    ag_tiles = {}
                    for hl in range(HPC):
                        for r in range(N_CORES):
                            ag = wk.tile([128, TBLK], BF16, tag="ag",
                                         name=f"ag{b}_{hl}_{r}_{_rep}",
                                         bufs=16)
                            nc.sync.dma_start(
                                ag[:], a2a_outs[(b, hl)][r, :, :])
                            ag_tiles[(hl, r)] = ag
                    for oc in range(4):
                        for t in range(2):
                            opst = ps.tile([128, 512], F32, tag="ps",
                                           name=f"ops{b}_{oc}_{t}_{_rep}")
                            for hl in range(HPC):
                                for r in range(N_CORES):
                                    g = HPC * r + hl
                                    nc.tensor.matmul(
                                        opst[:],
                                        ag_tiles[(hl, r)][:,
                                                          128 * t:
                                                          128 * (t + 1)],
                                        wd_sb[:, g, 512 * oc:512 * (oc + 1)],
                                        start=(hl == 0 and r == 0),
                                        stop=False)
                            nc.tensor.matmul(
                                opst[:], ones_row[:],
                                bdr_sb[:, 512 * oc:512 * (oc + 1)],
                                start=False, stop=True)
                            os_t = wk.tile([128, 512], F32, tag="os",
                                           name=f"os{b}_{oc}_{t}_{_rep}",
                                           bufs=2)
                            nc.scalar.activation(os_t[:], opst[:], Act.Copy)
                            nc.sync.dma_start(
                                out.ap()[TBLK * b + 128 * t:
                                         TBLK * b + 128 * (t + 1),
                                         512 * oc:512 * (oc + 1)],
                                os_t[:])
    nc.compile()
    return nc


def _get_nc(repeat=1):
    if repeat not in _NC:
        _NC[repeat] = _build(repeat)
    return _NC[repeat]


def _host_prep(hidden_states, attention_mask, position_ids, W_qkv, b_qkv,
               W_dense, b_dense):
    bf16 = ml_dtypes.bfloat16
    x = np.asarray(hidden_states, dtype=np.float32)
    am = np.asarray(attention_mask, dtype=np.float32)
    pos = np.asarray(position_ids)
    W_qkv = np.asarray(W_qkv, dtype=np.float32)
    b_qkv = np.asarray(b_qkv, dtype=np.float32)
    W_dense = np.asarray(W_dense, dtype=np.float32)
    b_dense = np.asarray(b_dense, dtype=np.float32)

    xT = np.ascontiguousarray(np.transpose(x, (0, 2, 1))).astype(bf16)
    wdT = np.ascontiguousarray(W_dense.T).astype(bf16)
    bd_row = np.ascontiguousarray(b_dense[None, :]).astype(bf16)

    inv_freq = (1.0 / (ROPE_BASE ** (np.arange(0, RD, 2, dtype=np.float32)
                                     / RD))).astype(np.float32)
    cosT = np.empty((B, RD, S), np.float32)
    sinT = np.empty((B, RD, S), np.float32)
    for b in range(B):
        freqs = pos[b].astype(np.float32)[:, None] * inv_freq[None, :]
        emb = np.concatenate([freqs, freqs], axis=1)        # (S, RD)
        cosT[b] = np.cos(emb).T
        sinT[b] = np.sin(emb).T
    cosT = cosT.astype(bf16)
    sinT = sinT.astype(bf16)

    kbias = np.empty((128, B * NKT), np.float32)
    for b in range(B):
        kbias[:, NKT * b:NKT * (b + 1)] = am[b, 0, 0].reshape(NKT, 128).T

    rT = np.zeros((RD, RD), np.float32)
    half = RD // 2
    rT[np.arange(half), np.arange(half) + half] = 1.0
    rT[np.arange(half) + half, np.arange(half)] = -1.0
    rT = rT.astype(bf16)

    in_maps = []
    for c in range(N_CORES):
        heads = [HPC * c + hl for hl in range(HPC)]
        wq = np.concatenate([W_qkv[384 * g:384 * g + 128] for g in heads])
        wkk = np.concatenate([W_qkv[384 * g + 128:384 * g + 256]
                              for g in heads])
        wv = np.concatenate([W_qkv[384 * g + 256:384 * g + 384]
                             for g in heads])
        bq_col = np.stack([b_qkv[384 * g:384 * g + 128] for g in heads],
                          axis=1)
        bk_col = np.stack([b_qkv[384 * g + 128:384 * g + 256] for g in heads],
                          axis=1)
        bv = np.concatenate([b_qkv[384 * g + 256:384 * g + 384]
                             for g in heads])
        bv_row2 = np.concatenate([bv, bv])[None, :]   # duplicated for packed
        in_maps.append({
            "xT": xT,
            "wqT": np.ascontiguousarray(wq.T).astype(bf16),
            "wkT": np.ascontiguousarray(wkk.T).astype(bf16),
            "wvT": np.ascontiguousarray(wv.T).astype(bf16),
            "wdT": wdT,
            "bq_col": np.ascontiguousarray(bq_col),
            "bk_col": np.ascontiguousarray(bk_col),
            "bv_row2": np.ascontiguousarray(bv_row2).astype(bf16),
            "bd_row": bd_row,
            "cosT": cosT,
            "sinT": sinT,
            "kbias": kbias,
            "rT": rT,
        })
    return in_maps


def run_sharded(trace=False, **inputs):
    """Run the bass kernel; returns (full_output, BassKernelResults)."""
    import time as _time
    nc = _get_nc()
    in_maps = _host_prep(**inputs)
    last_err = None
    for attempt in range(4):
        try:
            res = bass_utils.run_bass_kernel_spmd(
                nc, in_maps, core_ids=list(range(N_CORES)), trace=trace)
            break
        except Exception as e:  # device occasionally wedged by a prior session
            msg = str(e)
            last_err = e
            if ("UNAVAILABLE" in msg or "UNRECOVERABLE" in msg
                    or "unrecoverable" in msg):
                _time.sleep(5.0 * (attempt + 1))
                continue
            raise
    else:
        raise last_err
    full = np.empty((B, S, H), np.float32)
    for c in range(N_CORES):
        shard = res.results[c]["out"]
        for b in range(B):
            full[b, TBLK * c:TBLK * (c + 1)] = shard[TBLK * b:TBLK * (b + 1)]
    return full, res


def kernel(**inputs):
    full, _ = run_sharded(trace=False, **inputs)
    return full
